# revision 10
# baseline (speedup 1.0000x reference)
"""Trainium2 Bass kernel for nn_Attention_28948079575569 (softmax pooling).

Computation (reference):
    u      = tanh(h @ W1^T + b1)                 [B, T, D]
    alphas = softmax_t(u @ W2^T)                 [B, T, D]
    out    = sum_{b,t} h * alphas                [D]

Distribution: data-parallel over batch across 8 NeuronCores (4 batches per
core); small weights replicated; each core emits a partial [D] sum which the
host adds (the cross-core reduction is 2KB — not worth a collective).

v2 design (vs v1 which PE-transposed h on device from a DMA-cast bf16 copy):
  * h is pre-transposed AND pre-tiled on the host into per-chunk-contiguous
    [chunk, 128, kk, t] bf16 blocks -> plain max-rate HWDGE loads, no PE
    transpose, no PSUM->SBUF copies, no SWDGE cast-DMA.
  * TCH=1024 token chunks; mm outputs span 2 PSUM banks; tanh/exp process
    FD=1024 per instruction to amortize ACT's ~352cyc fixed overhead.
  * numerator uses fused vector.scalar_tensor_tensor with accum_out (one DVE
    op instead of mul+reduce). NOTE: tensor_tensor_reduce compiles + passes
    CoreSim but aborts on this HW/runtime path; scalar_tensor_tensor with
    op0=mult/scalar=1.0 is the HW-safe equivalent.
  * one-chunk software pipeline skew: PE stream is mm1(c) then mm2(c-1), so
    PE never waits for tanh(c)/exp(c-1) (ACT runs concurrently). The For_i
    back-edge keeps the pipeline full (DMA prefetch + PSUM pools carry over),
    so marginal per-iteration cost is bubble-free steady state.
  * M=128 matmuls, no tile_position col-split (ct=False): col-group MMs do
    not fully overlap their moving streams on HW, so a 2x64 split doubles PE
    streaming time (measured: ct=True ~292us vs ct=False ~210us).
  * optional fp8e4 DoubleRow paths for mm1/mm2 exist but are OFF: weight
    quantization to e4m3 alone costs rel_err ~0.028-0.038 (coherent across
    tokens, no averaging), over the 2e-2 gate. bf16 everywhere: ~0.0025.

Per-core per-chunk engine budget (bf16, warm, cost-model): PE 14.2us (the
bottleneck: 64 MMs x N=512 @ 2.4GHz is the streaming roofline), ACT 9.1us,
DVE 4.6us, DMA ~3us -> ideal ~218us for 16 chunks; HW measures ~210us.

exp needs no max-subtraction: |s| <= ||u||*||W2_row|| is bounded (~26 worst
case since |u|<1 via tanh), far below f32 overflow.
"""
import numpy as np
import ml_dtypes

import concourse.bacc as bacc
import concourse.bass as bass
import concourse.tile as tile
from concourse import bass_utils, mybir

F32 = mybir.dt.float32
BF16 = mybir.dt.bfloat16
FP8 = mybir.dt.float8e4
Act = mybir.ActivationFunctionType
Alu = mybir.AluOpType
DR = mybir.MatmulPerfMode.DoubleRow

B, T, D = 32, 4096, 512
N_CORES = 8
B_LOC = B // N_CORES         # batches per core
TCH = 1024                   # t-chunk size
NCH = T // TCH               # chunks per batch
NCT = B_LOC * NCH            # chunks per core
NT = D // 128                # 128-partition tiles per feature dim
NHF = TCH // 512             # 512-wide PSUM halves per chunk

# mode: "bf16" | "fp8mm2" | "fp8" (mm1+mm2 fp8, numerator bf16 h)
#       | "fp8all" (numerator h fp8 too; only fp8 h is loaded)
MODE = "bf16"
WSCALE = 32.0                # fp8 weight pre-scale (folded back in ACT scale)


def build(repeat=1, loop=False, mode=MODE, bufs=3, ct=False):
    # ct: split each 128-col weight into 2x64 col-groups (tile_position).
    # Col-group MMs do NOT overlap their moving streams on HW (v1 calib:
    # TimelineSim serialized model matched HW 494us) -> ct=True doubles PE
    # streaming; keep False and rely on LDW pull-ahead to hide weight loads.
    fp8_mm1 = mode in ("fp8", "fp8all", "fp8mm1")
    fp8_mm2 = mode in ("fp8mm2", "fp8", "fp8all")
    h_bf = mode != "fp8all"          # load bf16 h for the numerator
    h_f8 = fp8_mm1                   # load fp8 h for mm1
    ws = WSCALE

    nc = bacc.Bacc("TRN2", target_bir_lowering=False, debug=False)
    if h_bf:
        hd = nc.dram_tensor("h", [NCT, 128, NT, TCH], BF16,
                            kind="ExternalInput")
    if h_f8:
        h8d = nc.dram_tensor("h8", [NCT, 128, 2, 2, TCH], FP8,
                             kind="ExternalInput")
    if fp8_mm1:
        w1d = nc.dram_tensor("w1t", [128, 2, 2, D], FP8, kind="ExternalInput")
    else:
        w1d = nc.dram_tensor("w1t", [128, NT, D], BF16, kind="ExternalInput")
    if fp8_mm2:
        w2d = nc.dram_tensor("w2t", [128, 2, 2, D], FP8, kind="ExternalInput")
    else:
        w2d = nc.dram_tensor("w2t", [128, NT, D], BF16, kind="ExternalInput")
    b1d = nc.dram_tensor("b1", [128, NT, 1], F32, kind="ExternalInput")
    out = nc.dram_tensor("out", [NT, 128], F32, kind="ExternalOutput")

    with tile.TileContext(nc) as tc:
        import contextlib
        stk = contextlib.ExitStack()
        wp = stk.enter_context(tc.tile_pool(name="wts", bufs=1))
        hp = stk.enter_context(tc.tile_pool(name="hT", bufs=bufs))
        h8p = stk.enter_context(tc.tile_pool(name="hT8", bufs=bufs))
        up_ = stk.enter_context(tc.tile_pool(name="u", bufs=2))
        pp_ = stk.enter_context(tc.tile_pool(name="P", bufs=2))
        qp_ = stk.enter_context(tc.tile_pool(name="q", bufs=2))
        znp = stk.enter_context(tc.tile_pool(name="zn", bufs=2))
        smp = stk.enter_context(tc.tile_pool(name="small", bufs=4))
        resp = stk.enter_context(tc.tile_pool(name="res", bufs=1))
        ps1p = stk.enter_context(tc.tile_pool(name="ps1", bufs=2, space="PSUM"))
        ps2p = stk.enter_context(tc.tile_pool(name="ps2", bufs=2, space="PSUM"))

        if fp8_mm1:
            w1_sb = wp.tile([128, 2, 2, D], FP8)
        else:
            w1_sb = wp.tile([128, NT, D], BF16)
        nc.sync.dma_start(w1_sb[:], w1d.ap()[:])
        if fp8_mm2:
            w2_sb = wp.tile([128, 2, 2, D], FP8)
        else:
            w2_sb = wp.tile([128, NT, D], BF16)
        nc.sync.dma_start(w2_sb[:], w2d.ap()[:])
        b1_sb = wp.tile([128, NT, 1], F32)
        nc.sync.dma_start(b1_sb[:], b1d.ap()[:])

        acc = resp.tile([128, NT], F32)
        nc.vector.memset(acc[:], 0.0)

        # ---- stage 1: load + mm1 + tanh for chunk c ----
        def stage1(c, state):
            if h_bf:
                hT = hp.tile([128, NT, TCH], BF16, tag="hT")
                nc.sync.dma_start(hT[:], hd.ap()[c])
                state["hT"][c] = hT
            if h_f8:
                hT8 = h8p.tile([128, 2, 2, TCH], FP8, tag="hT8")
                nc.sync.dma_start(hT8[:], h8d.ap()[c])
                state["hT8"][c] = hT8
            # u layout: [128, kk2, ko2, TCH] (fp8 mm2) or [128, NT, TCH]
            if fp8_mm2:
                u_sb = up_.tile([128, 2, 2, TCH], FP8, tag="u")
            else:
                u_sb = up_.tile([128, NT, TCH], BF16, tag="u")
            state["u"][c] = u_sb
            for mm in range(NT):
                ps = ps1p.tile([128, TCH], F32, tag="ps1")
                if fp8_mm1:
                    for kk in range(2):
                        for hh in range(NHF):
                            nc.tensor.matmul(
                                ps[:, bass.ts(hh, 512)],
                                w1_sb[:, kk, :, bass.ds(mm * 128, 128)],
                                state["hT8"][c][:, kk, :, bass.ts(hh, 512)],
                                start=(kk == 0), stop=(kk == 1),
                                perf_mode=DR, skip_group_check=True)
                elif ct:
                    for kk in range(NT):
                        for hh in range(NHF):
                            for j in range(2):
                                nc.tensor.matmul(
                                    ps[64 * j:64 * j + 64, bass.ts(hh, 512)],
                                    w1_sb[:, kk,
                                          bass.ds(mm * 128 + 64 * j, 64)],
                                    state["hT"][c][:, kk, bass.ts(hh, 512)],
                                    start=(kk == 0), stop=(kk == NT - 1),
                                    tile_position=(0, 64 * j),
                                    skip_group_check=True)
                else:
                    for kk in range(NT):
                        for hh in range(NHF):
                            nc.tensor.matmul(
                                ps[:, bass.ts(hh, 512)],
                                w1_sb[:, kk, bass.ds(mm * 128, 128)],
                                state["hT"][c][:, kk, bass.ts(hh, 512)],
                                start=(kk == 0), stop=(kk == NT - 1),
                                skip_group_check=True)
                udst = (u_sb[:, mm // 2, mm % 2, :] if fp8_mm2
                        else u_sb[:, mm, :])
                nc.scalar.activation(
                    udst, ps[:], Act.Tanh,
                    bias=b1_sb[:, mm, :],
                    scale=(1.0 / ws if fp8_mm1 else 1.0))

        # ---- stage 2: mm2 + exp + numerator for chunk c ----
        def stage2(c, state):
            b, ch = divmod(c, NCH)
            if ch == 0:
                state["Z"] = znp.tile([128, NT, NCH], F32, tag="Zc", name="Zc")
                state["N"] = znp.tile([128, NT, NCH], F32, tag="Nc", name="Nc")
            Zc, Ncc = state["Z"], state["N"]
            u_sb = state["u"].pop(c)
            P_sb = pp_.tile([128, NT, TCH], BF16, tag="P")
            for me in range(NT):
                ps = ps2p.tile([128, TCH], F32, tag="ps2")
                if fp8_mm2:
                    for kk in range(2):
                        for hh in range(NHF):
                            nc.tensor.matmul(
                                ps[:, bass.ts(hh, 512)],
                                w2_sb[:, kk, :, bass.ds(me * 128, 128)],
                                u_sb[:, kk, :, bass.ts(hh, 512)],
                                start=(kk == 0), stop=(kk == 1),
                                perf_mode=DR, skip_group_check=True)
                elif ct:
                    for kk in range(NT):
                        for hh in range(NHF):
                            for j in range(2):
                                nc.tensor.matmul(
                                    ps[64 * j:64 * j + 64, bass.ts(hh, 512)],
                                    w2_sb[:, kk,
                                          bass.ds(me * 128 + 64 * j, 64)],
                                    u_sb[:, kk, bass.ts(hh, 512)],
                                    start=(kk == 0), stop=(kk == NT - 1),
                                    tile_position=(0, 64 * j),
                                    skip_group_check=True)
                else:
                    for kk in range(NT):
                        for hh in range(NHF):
                            nc.tensor.matmul(
                                ps[:, bass.ts(hh, 512)],
                                w2_sb[:, kk, bass.ds(me * 128, 128)],
                                u_sb[:, kk, bass.ts(hh, 512)],
                                start=(kk == 0), stop=(kk == NT - 1),
                                skip_group_check=True)
                nc.scalar.activation(
                    P_sb[:, me, :], ps[:], Act.Exp,
                    scale=(1.0 / ws if fp8_mm2 else 1.0),
                    accum_out=Zc[:, me, ch:ch + 1])
            hnum = (state["hT"].pop(c) if h_bf else state["hT8"].pop(c))
            for me in range(NT):
                hsl = (hnum[:, me, :] if h_bf
                       else hnum[:, me // 2, me % 2, :])
                q = qp_.tile([128, TCH], BF16, tag="q")
                nc.vector.scalar_tensor_tensor(
                    out=q[:], in0=hsl, scalar=1.0, in1=P_sb[:, me, :],
                    op0=Alu.mult, op1=Alu.mult,
                    accum_out=Ncc[:, me, ch:ch + 1])
            if ch == NCH - 1:
                for me in range(NT):
                    zb = smp.tile([128, 1], F32, tag="zb")
                    nc.vector.tensor_reduce(
                        zb[:], Zc[:, me, :], axis=mybir.AxisListType.X,
                        op=Alu.add)
                    rz = smp.tile([128, 1], F32, tag="rz")
                    nc.vector.reciprocal(rz[:], zb[:])
                    nb = smp.tile([128, 1], F32, tag="nb")
                    nc.vector.tensor_reduce(
                        nb[:], Ncc[:, me, :], axis=mybir.AxisListType.X,
                        op=Alu.add)
                    pr = smp.tile([128, 1], F32, tag="pr")
                    nc.vector.tensor_mul(pr[:], nb[:], rz[:])
                    nc.vector.tensor_add(
                        acc[:, me:me + 1], acc[:, me:me + 1], pr[:])

        def body():
            state = {"hT": {}, "hT8": {}, "u": {}}
            for c in range(NCT + 1):
                if c < NCT:
                    stage1(c, state)
                if c > 0:
                    stage2(c - 1, state)

        if loop and repeat > 1:
            with tc.For_i(0, repeat, 1):
                body()
        else:
            for _rep in range(repeat):
                body()

        nc.sync.dma_start(out.ap().rearrange("i p -> p i"), acc[:])
        stk.close()

    nc.compile()
    return nc


def _tile_hT(hcore, dt):
    """[B_LOC, T, D] f32 -> [NCT, 128, NT, TCH] per-chunk contiguous."""
    x = hcore.astype(dt)                                   # cast first (fast)
    x = x.reshape(B_LOC, NCH, TCH, NT, 128)                # [b, ch, t, kk, p]
    x = x.transpose(0, 1, 4, 3, 2)                         # [b, ch, p, kk, t]
    return np.ascontiguousarray(x).reshape(NCT, 128, NT, TCH)


def _tile_hT8(hcore, dt):
    """[B_LOC, T, D] f32 -> [NCT, 128, 2, 2, TCH]; d = kk*256 + ko*128 + p."""
    x = hcore.astype(dt)
    x = x.reshape(B_LOC, NCH, TCH, 2, 2, 128)              # [b,ch,t,kk,ko,p]
    x = x.transpose(0, 1, 5, 3, 4, 2)                      # [b,ch,p,kk,ko,t]
    return np.ascontiguousarray(x).reshape(NCT, 128, 2, 2, TCH)


def _w_bf16(W):
    """W [e,d] -> lhsT blocks [128, kk, e] bf16 (w[p,kk,e] = W[e, kk*128+p])."""
    WT = np.ascontiguousarray(np.asarray(W, np.float32).T)      # [d, e]
    return np.ascontiguousarray(
        WT.reshape(NT, 128, D).transpose(1, 0, 2)).astype(ml_dtypes.bfloat16)


def _w_fp8(W, ws):
    """W [e,d] -> DoubleRow lhsT [128, kk2, ko2, e]; d = kk*256+ko*128+p."""
    WT = np.ascontiguousarray(np.asarray(W, np.float32).T) * ws  # [d, e]
    x = WT.reshape(2, 2, 128, D).transpose(2, 0, 1, 3)           # [p,kk,ko,e]
    return np.ascontiguousarray(x).astype(ml_dtypes.float8_e4m3)


def make_in_maps(hidden_states, W1, b1, W2, mode=MODE):
    fp8_mm1 = mode in ("fp8", "fp8all", "fp8mm1")
    fp8_mm2 = mode in ("fp8mm2", "fp8", "fp8all")
    h_bf = mode != "fp8all"
    h = np.asarray(hidden_states, dtype=np.float32).reshape(
        N_CORES, B_LOC, T, D)
    b1c = np.ascontiguousarray(
        np.asarray(b1, np.float32).reshape(NT, 128).T).reshape(128, NT, 1)
    w1m = _w_fp8(W1, WSCALE) if fp8_mm1 else _w_bf16(W1)
    w2m = _w_fp8(W2, WSCALE) if fp8_mm2 else _w_bf16(W2)
    maps = []
    for i in range(N_CORES):
        m = {"w1t": w1m, "w2t": w2m, "b1": b1c}
        if h_bf:
            m["h"] = _tile_hT(h[i], ml_dtypes.bfloat16)
        if fp8_mm1:
            m["h8"] = _tile_hT8(h[i], ml_dtypes.float8_e4m3)
        maps.append(m)
    return maps


_NC_CACHE = {}


def _get_nc():
    if "nc" not in _NC_CACHE:
        _NC_CACHE["nc"] = build(repeat=1)
    return _NC_CACHE["nc"]


def kernel(hidden_states, W1, b1, W2):
    assert np.asarray(hidden_states).shape == (B, T, D)
    in_maps = make_in_maps(hidden_states, W1, b1, W2)
    nc = _get_nc()
    last_err = None
    for attempt in range(3):
        try:
            res = bass_utils.run_bass_kernel_spmd(
                nc, in_maps, core_ids=list(range(N_CORES)), trace=False)
            break
        except Exception as e:   # transient axon/PJRT hiccups
            last_err = e
            import time
            time.sleep(2.0 * (attempt + 1))
    else:
        raise last_err
    total = np.zeros(D, np.float64)
    for r in res.results:
        total += r["out"].reshape(D).astype(np.float64)
    return total.astype(np.float32)
